# revision 22
# baseline (speedup 1.0000x reference)
"""Trainium2 Bass kernel for the MemoryModule problem.

Per batch element b (8 of them, one per NeuronCore):
    mk = memory_keys[:, b]  viewed as (Ck=128, M=8192)   [M = T*H*W]
    mv = memory_values[:, b] viewed as (Cv=512, M)
    qk = query_key[b]       viewed as (Ck=128, N=1024)   [N = H*W]
    S  = qk^T @ mk          (N, M)
    P  = softmax(S, axis=-1)
    mem = (P @ mv^T)^T      (Cv, N)
    out[b] = concat([query_value[b], mem], channel axis)

Device dataflow (all transposes done on host / by layout, none on chip):
    - S^T computed directly: S^T tile (128 m, n) = matmul(lhsT=mk_tile, rhs=qk)
      with fp16 inputs (1 cyc/row on the PE vs 4 for fp32).
    - exp on ScalarE (no max subtraction: |S| <~ 70, exp fits fp32/bf16 range),
      written as bf16 P^T tiles.
    - PV: matmul(lhsT=P^T chunk (m,128n), rhs=mv^T tile (m,512c)) in bf16,
      accumulated over the 64 m tiles in PSUM (N=512 = exactly one bank).
      mv^T layout comes straight from a host-side transpose.
    - softmax denominator: VectorE (otherwise idle) accumulates the P^T tiles
      into an SBUF accumulator; at the end of each n-half, 4 tiny PE matmuls
      against a ones vector reduce the 128 partition-partials per n column.
    - normalize with reciprocal + per-partition scalar multiply, DMA out
      as mem^T (N, Cv); host transposes back.

n is processed in halves of 512 so PSUM fits: 3 S^T banks (triple-buffered)
+ 4 PV accumulators + 1 denominator bank = 8.
"""

import os

import numpy as np
import ml_dtypes

T, B, Ck, Cv, H, W = 8, 8, 128, 512, 32, 32
HW = H * W            # 1024  (n dimension)
M = T * HW            # 8192  (memory / contraction dimension)
MT = M // 128         # 64 m-tiles
NQ = 2                # process n in halves
NQS = HW // NQ        # 512 columns of S^T per half
NCH = NQS // 128      # 4 PV accumulators per half
N_CORES = 8

# "f16": fp16 QK matmul (1 cyc/row, ~5e-4 input rounding)
# "f32r": fp32r QK matmul (1 cyc/row, hardware-reduced fp32 precision)
# "f32": exact fp32 QK matmul (4 cyc/row, slow)
QK_MODE = os.environ.get("KERNEL_QK_MODE", "f16")
# >1: repeat the full compute (incl. input DMAs) inside one NEFF via a
# hardware For_i loop, for HW timing via wall-clock deltas. Output is
# identical (rewritten each iteration).
LOOP = int(os.environ.get("KERNEL_LOOP", "1"))
# timing diagnostics: "full" | "dma" (loop only DMAs) | "compute" (DMAs
# hoisted out of the loop, loop only compute)
MODE = os.environ.get("KERNEL_MODE", "full")

_CACHE = {}
LAST_RESULTS = None


def _build_nc(qk_mode, loop=1, mode="full"):
    import concourse.tile as tile
    import concourse.mybir as mybir
    from concourse import bacc

    f32 = mybir.dt.float32
    bf16 = mybir.dt.bfloat16
    f16 = mybir.dt.float16
    qk_dt = {"f16": f16, "f32r": f32, "f32": f32}[qk_mode]

    nc = bacc.Bacc()

    qk_d = nc.dram_tensor("qk", [Ck, HW], qk_dt, kind="ExternalInput")
    mk_d = nc.dram_tensor("mk", [Ck, M], qk_dt, kind="ExternalInput")
    mv_d = nc.dram_tensor("mv", [M, Cv], bf16, kind="ExternalInput")
    out_d = nc.dram_tensor("out", [HW, Cv], f32, kind="ExternalOutput")

    mv_tiled = mv_d.rearrange("(mt p) c -> mt p c", p=128)  # (64, 128, 512)

    Exp = mybir.ActivationFunctionType.Exp
    AluOp = mybir.AluOpType

    def emit_dma(nc, tc, big):
        qk_sb = big.tile([Ck, HW], qk_dt, tag="qk_sb", name="qk_sb")
        nc.sync.dma_start(qk_sb[:], qk_d[:])
        mk_sb = big.tile([Ck, M], qk_dt, tag="mk_sb", name="mk_sb")
        # split so the first S^T matmuls don't wait for the whole tensor
        for i in range(8):
            nc.sync.dma_start(
                mk_sb[:, i * HW : (i + 1) * HW], mk_d[:, i * HW : (i + 1) * HW]
            )
        mv_sb = big.tile([128, MT, Cv], bf16, tag="mv_sb", name="mv_sb")
        for m in range(MT):
            nc.sync.dma_start(mv_sb[:, m], mv_tiled[m])
        return qk_sb, mk_sb, mv_sb

    def body(nc, tc, big, ptp, accp, outp, smallp, stp, pvp, dnp, tiles):
        qk_sb, mk_sb, mv_sb = tiles
        ones_sb = big.tile([128, 1], f32, tag="ones_sb", name="ones_sb")
        nc.vector.memset(ones_sb[:], 1.0)

        def mm_cast(ap):
            if qk_mode == "f32r":
                return ap.bitcast(mybir.dt.float32r)
            return ap

        for q in range(NQ):
            pv = [
                pvp.tile([128, NQS], f32, tag=f"pv{i}", name=f"pv_q{q}_{i}")
                for i in range(NCH)
            ]
            acc = accp.tile([128, NQS], f32, tag="acc", name=f"acc_q{q}")
            sts = {}
            pts = {}

            def emit_st(m, q=q, sts=sts):
                st = stp.tile([128, NQS], f32, tag="st", name=f"st_q{q}_m{m}")
                nc.tensor.matmul(
                    st[:],
                    mm_cast(mk_sb[:, m * 128 : (m + 1) * 128]),
                    mm_cast(qk_sb[:, q * NQS : (q + 1) * NQS]),
                    start=True,
                    stop=True,
                )
                sts[m] = st

            def emit_exp(m, q=q, sts=sts, pts=pts):
                pt = ptp.tile([128, NQS], bf16, tag="pt", name=f"pt_q{q}_m{m}")
                nc.scalar.activation(pt[:], sts.pop(m)[:], Exp)
                pts[m] = pt

            def emit_acc(m, acc=acc, pts=pts):
                # VectorE: accumulate exp tiles for the softmax denominator
                if m == 0:
                    nc.vector.tensor_copy(acc[:], pts[m][:])
                else:
                    nc.vector.tensor_tensor(acc[:], acc[:], pts[m][:], AluOp.add)

            # software pipeline: PE always has the next S^T ready, ACT runs
            # two tiles ahead of the PV consumers so PV's LDWEIGHTS never
            # waits on an unsatisfied semaphore
            emit_st(0)
            emit_st(1)
            emit_exp(0)
            emit_st(2)
            emit_exp(1)
            emit_st(3)
            for m in range(MT):
                ptm = pts[m]
                for nch in range(NCH):
                    nc.tensor.matmul(
                        pv[nch][:],
                        ptm[:, nch * 128 : (nch + 1) * 128],
                        mv_sb[:, m],
                        start=(m == 0),
                        stop=(m == MT - 1),
                    )
                if m + 2 < MT:
                    emit_exp(m + 2)
                emit_acc(m)
                del pts[m]
                if m + 4 < MT:
                    emit_st(m + 4)

            # denominator: reduce acc over partitions with 4 tiny matmuls
            dn = dnp.tile([128, NCH], f32, tag="dn", name=f"dn_q{q}")
            for nch in range(NCH):
                nc.tensor.matmul(
                    dn[:, nch : nch + 1],
                    acc[:, nch * 128 : (nch + 1) * 128],
                    ones_sb[:],
                    start=True,
                    stop=True,
                )
            recip = smallp.tile([128, NCH], f32, tag="recip", name=f"recip_q{q}")
            nc.vector.reciprocal(recip[:], dn[:])
            for nch in range(NCH):
                o = outp.tile([128, Cv], f32, tag="o", name=f"o_q{q}_{nch}")
                # split the tail normalize across DVE and ACT so the final
                # PSUM evacuation halves in wall-clock
                if nch % 2 == 0:
                    nc.vector.tensor_scalar_mul(
                        o[:], pv[nch][:], recip[:, nch : nch + 1]
                    )
                else:
                    nc.scalar.activation(
                        o[:],
                        pv[nch][:],
                        mybir.ActivationFunctionType.Copy,
                        scale=recip[:, nch : nch + 1],
                    )
                n0 = q * NQS + nch * 128
                nc.sync.dma_start(out_d[n0 : n0 + 128, :], o[:])

    with tile.TileContext(nc) as tc:
        with (
            tc.tile_pool(name="big", bufs=1) as big,
            tc.tile_pool(name="ptp", bufs=5) as ptp,
            tc.tile_pool(name="accp", bufs=2) as accp,
            tc.tile_pool(name="outp", bufs=3) as outp,
            tc.tile_pool(name="smallp", bufs=2) as smallp,
            tc.tile_pool(name="stp", bufs=3, space="PSUM") as stp,
            tc.tile_pool(name="pvp", bufs=1, space="PSUM") as pvp,
            tc.tile_pool(name="dnp", bufs=1, space="PSUM") as dnp,
        ):
            if mode == "mmonly" and loop > 1:
                # pure PE stream: same LDW+MM pair count/shapes as the real
                # kernel, but no ACT/DVE in the loop (weights from a fixed
                # dummy tile)
                tiles = emit_dma(nc, tc, big)
                qk_sb, mk_sb, mv_sb = tiles
                dummy_pt = big.tile([128, NQS], bf16, tag="dummy_pt", name="dummy_pt")
                nc.vector.memset(dummy_pt[:], 0.001)
                with tc.For_i(0, loop, 1):
                    for q in range(NQ):
                        pv = [
                            pvp.tile([128, NQS], f32, tag=f"pv{i}", name=f"mm_pv_q{q}_{i}")
                            for i in range(NCH)
                        ]
                        for m in range(MT):
                            st = stp.tile([128, NQS], f32, tag="st", name=f"mm_st_q{q}_m{m}")
                            nc.tensor.matmul(
                                st[:],
                                mk_sb[:, m * 128 : (m + 1) * 128],
                                qk_sb[:, q * NQS : (q + 1) * NQS],
                                start=True,
                                stop=True,
                            )
                            for nch in range(NCH):
                                nc.tensor.matmul(
                                    pv[nch][:],
                                    dummy_pt[:, nch * 128 : (nch + 1) * 128],
                                    mv_sb[:, m],
                                    start=(m == 0),
                                    stop=(m == MT - 1),
                                )
                        for nch in range(NCH):
                            o = outp.tile([128, Cv], f32, tag="o", name=f"mm_o_q{q}_{nch}")
                            nc.vector.tensor_copy(o[:], pv[nch][:])
                            nc.sync.dma_start(
                                out_d[(q * NCH + nch) * 128 : (q * NCH + nch + 1) * 128, :],
                                o[:],
                            )
            elif mode == "compute" and loop > 1:
                tiles = emit_dma(nc, tc, big)
                with tc.For_i(0, loop, 1):
                    body(nc, tc, big, ptp, accp, outp, smallp, stp, pvp, dnp, tiles)
            elif mode == "dma" and loop > 1:
                with tc.For_i(0, loop, 1):
                    emit_dma(nc, tc, big)
                    # include the output-store traffic too
                    for j in range(8):
                        o = outp.tile([128, Cv], f32, tag="o", name=f"o_{j}")
                        nc.vector.memset(o[:], float(j))
                        nc.sync.dma_start(out_d[j * 128 : (j + 1) * 128, :], o[:])
            else:
                loop_ctx = tc.For_i(0, loop, 1) if loop > 1 else None
                with (loop_ctx if loop_ctx is not None else _null()):
                    tiles = emit_dma(nc, tc, big)
                    body(nc, tc, big, ptp, accp, outp, smallp, stp, pvp, dnp, tiles)

    nc.finalize()
    return nc


class _null:
    def __enter__(self):
        return None

    def __exit__(self, *a):
        return False


def _get_nc():
    key = ("nc", QK_MODE, LOOP, MODE)
    if key not in _CACHE:
        _CACHE[key] = _build_nc(QK_MODE, LOOP, MODE)
    return _CACHE[key]


def _prep_core_inputs(memory_keys, memory_values, query_key, b):
    np_qk_dt = np.float16 if QK_MODE == "f16" else np.float32
    qk = np.ascontiguousarray(query_key[b].reshape(Ck, HW)).astype(np_qk_dt)
    mk = np.ascontiguousarray(
        memory_keys[:, b].transpose(1, 0, 2, 3).reshape(Ck, M)
    ).astype(np_qk_dt)
    mv = np.ascontiguousarray(
        memory_values[:, b].transpose(0, 2, 3, 1).reshape(M, Cv)
    ).astype(ml_dtypes.bfloat16)
    return {"qk": qk, "mk": mk, "mv": mv}


def kernel(memory_keys, memory_values, query_key, query_value):
    global LAST_RESULTS
    from concourse.bass_utils import run_bass_kernel_spmd

    memory_keys = np.asarray(memory_keys, dtype=np.float32)
    memory_values = np.asarray(memory_values, dtype=np.float32)
    query_key = np.asarray(query_key, dtype=np.float32)
    query_value = np.asarray(query_value, dtype=np.float32)

    in_maps = [
        _prep_core_inputs(memory_keys, memory_values, query_key, b)
        for b in range(N_CORES)
    ]
    res = run_bass_kernel_spmd(_get_nc(), in_maps, core_ids=list(range(N_CORES)))
    LAST_RESULTS = res

    mem = np.stack(
        [res.results[b]["out"].T.reshape(Cv, H, W) for b in range(N_CORES)]
    ).astype(np.float32)
    return np.concatenate([query_value, mem], axis=1)
